# revision 9
# baseline (speedup 1.0000x reference)
"""IF spiking-neuron scan (charge / fire / hard-reset) on 8 Trainium2 cores.

Reference recurrence over t (elementwise on every [B, N] element):
    v = v + x_t
    s = (v - 1.0 >= 0)          # spike, 0.0/1.0
    v = (1 - s) * v             # hard reset to 0

Sharding: pure data parallel over the B*N = 262144 element dimension;
each of the 8 cores owns 32768 element chains [T=64, 32768] with zero
communication. Per core the elements live in SBUF as a [128, 256] f32
state tile; the 64-step scan runs locally.

Engine split per timestep (per core):
  DVE   : u = v + x_t            (tensor_tensor add)
          v = (u < 1) * u        (scalar_tensor_tensor fused compare+mult)
  GPSIMD: s = (u >= 1)           (tensor_scalar is_ge -> output tile)
The v-chain (add -> reset) stays on one engine so the serial dependency
never pays a cross-engine semaphore hop; the spike compare branches off.
DMA: timesteps are blocked 8-at-a-time into 1 MiB HWDGE transfers.
"""

import numpy as np

import concourse.bass as bass
import concourse.tile as tile
from concourse import bacc, mybir
from concourse.bass_utils import run_bass_kernel_spmd

T = 64
B = 32
N = 8192
NCORES = 8
PERCORE = (B * N) // NCORES  # 32768 element chains per core
P = 128                      # SBUF partitions
F = PERCORE // P             # 256 elements per partition
TB = 8                       # timesteps per DMA block
NBLK = T // TB

V_TH = 1.0

_NC_CACHE = {}


def build_nc(spike_engine="gpsimd", u_bufs=T + 1, tb=TB):
    nblk = T // tb
    # Bacc (not raw Bass): its compile() splits multi-wait sync conditions
    # into nop/event-semaphore prefixes — walrus accepts at most one sync
    # wait per hardware instruction.
    nc = bacc.Bacc("TRN2", target_bir_lowering=False, debug=False)
    x = nc.dram_tensor("x", [T, PERCORE], mybir.dt.float32, kind="ExternalInput").ap()
    y = nc.dram_tensor("y", [T, PERCORE], mybir.dt.float32, kind="ExternalOutput").ap()

    # [T, P*F] -> [P, T, F]: per partition, each timestep is a contiguous
    # 1 KiB run in DRAM.
    xr = x.rearrange("t (p f) -> p t f", p=P)
    yr = y.rearrange("t (p f) -> p t f", p=P)

    with tile.TileContext(nc) as tc:
        with (
            tc.tile_pool(name="xin", bufs=3) as xpool,
            # every output block gets its own slot: no slot-reuse waits ever
            # land on the Pool is_ge instructions
            tc.tile_pool(name="sout", bufs=nblk) as spool,
            # one u slot per timestep: the DVE add never waits on a Pool
            # reader freeing a slot
            tc.tile_pool(name="u", bufs=u_bufs) as upool,
            tc.tile_pool(name="v", bufs=1) as vpool,
        ):
            spike_eng = getattr(nc, spike_engine)
            v = vpool.tile([P, F], mybir.dt.float32)
            nc.vector.memset(v[:], 0.0)
            for blk in range(nblk):
                xt = xpool.tile([P, tb * F], mybir.dt.float32)
                nc.sync.dma_start(xt[:], xr[:, blk * tb:(blk + 1) * tb, :])
                st = spool.tile([P, tb * F], mybir.dt.float32)
                for ti in range(tb):
                    xs = xt[:, ti * F:(ti + 1) * F]
                    ss = st[:, ti * F:(ti + 1) * F]
                    u = upool.tile([P, F], mybir.dt.float32)
                    nc.vector.tensor_add(u[:], v[:], xs)
                    spike_eng.tensor_scalar(
                        ss, u[:], V_TH, None, mybir.AluOpType.is_ge
                    )
                    nc.vector.scalar_tensor_tensor(
                        v[:], u[:], V_TH, u[:],
                        mybir.AluOpType.is_lt, mybir.AluOpType.mult,
                    )
                nc.sync.dma_start(yr[:, blk * tb:(blk + 1) * tb, :], st[:])
    nc.compile()
    return nc


def _get_nc():
    if "nc" not in _NC_CACHE:
        _NC_CACHE["nc"] = build_nc()
    return _NC_CACHE["nc"]


def run_sharded(x_seq, trace=False, nc=None, **kwargs):
    if nc is None:
        nc = _get_nc()
    x2 = np.ascontiguousarray(np.asarray(x_seq, dtype=np.float32)).reshape(T, B * N)
    in_maps = [
        {"x": np.ascontiguousarray(x2[:, c * PERCORE:(c + 1) * PERCORE])}
        for c in range(NCORES)
    ]
    res = run_bass_kernel_spmd(nc, in_maps, list(range(NCORES)), trace=trace, **kwargs)
    out = np.concatenate(
        [np.asarray(res.results[c]["y"]) for c in range(NCORES)], axis=1
    )
    return out.reshape(T, B, N).astype(np.float32, copy=False), res


def kernel(x_seq):
    out, _ = run_sharded(x_seq)
    return out


# revision 10
# speedup vs baseline: 3.0077x; 3.0077x over previous
"""IF spiking-neuron scan (charge / fire / hard-reset) on 8 Trainium2 cores.

Reference recurrence over t (elementwise on every [B, N] element):
    v = v + x_t
    s = (v - 1.0 >= 0)          # spike, 0.0/1.0
    v = (1 - s) * v             # hard reset to 0

Sharding: pure data parallel over the B*N = 262144 element dimension;
each of the 8 cores owns 32768 element chains [T=64, 32768] with zero
communication. Per core the elements live in SBUF as a [128, 256] f32
state tile; the 64-step scan runs locally.

Engine split per timestep (per core):
  DVE   : u = v + x_t            (tensor_tensor add)
          v = (u < 1) * u        (scalar_tensor_tensor fused compare+mult)
  GPSIMD: s = (u >= 1)           (tensor_scalar is_ge -> output tile)
The v-chain (add -> reset) stays on one engine so the serial dependency
never pays a cross-engine semaphore hop; the spike compare branches off.
DMA: timesteps are blocked 8-at-a-time into 1 MiB HWDGE transfers.
"""

import numpy as np

import concourse.bass as bass
import concourse.tile as tile
from concourse import bacc, mybir
from concourse.bass_utils import run_bass_kernel_spmd

T = 64
B = 32
N = 8192
NCORES = 8
PERCORE = (B * N) // NCORES  # 32768 element chains per core
P = 128                      # SBUF partitions
F = PERCORE // P             # 256 elements per partition
TB = 8                       # timesteps per DMA block
NBLK = T // TB

V_TH = 1.0

_NC_CACHE = {}


def build_nc(spike_engine="vector", u_bufs=4, tb=TB):
    nblk = T // tb
    # Bacc (not raw Bass): its compile() splits multi-wait sync conditions
    # into nop/event-semaphore prefixes — walrus accepts at most one sync
    # wait per hardware instruction.
    nc = bacc.Bacc("TRN2", target_bir_lowering=False, debug=False)
    x = nc.dram_tensor("x", [T, PERCORE], mybir.dt.float32, kind="ExternalInput").ap()
    y = nc.dram_tensor("y", [T, PERCORE], mybir.dt.float32, kind="ExternalOutput").ap()

    # [T, P*F] -> [P, T, F]: per partition, each timestep is a contiguous
    # 1 KiB run in DRAM.
    xr = x.rearrange("t (p f) -> p t f", p=P)
    yr = y.rearrange("t (p f) -> p t f", p=P)

    with tile.TileContext(nc) as tc:
        with (
            tc.tile_pool(name="xin", bufs=3) as xpool,
            # every output block gets its own slot: no slot-reuse waits ever
            # land on the Pool is_ge instructions
            tc.tile_pool(name="sout", bufs=nblk) as spool,
            # one u slot per timestep: the DVE add never waits on a Pool
            # reader freeing a slot
            tc.tile_pool(name="u", bufs=u_bufs) as upool,
            tc.tile_pool(name="v", bufs=1) as vpool,
        ):
            spike_eng = getattr(nc, spike_engine)
            v = vpool.tile([P, F], mybir.dt.float32)
            nc.vector.memset(v[:], 0.0)
            for blk in range(nblk):
                xt = xpool.tile([P, tb * F], mybir.dt.float32)
                nc.sync.dma_start(xt[:], xr[:, blk * tb:(blk + 1) * tb, :])
                st = spool.tile([P, tb * F], mybir.dt.float32)
                for ti in range(tb):
                    xs = xt[:, ti * F:(ti + 1) * F]
                    ss = st[:, ti * F:(ti + 1) * F]
                    u = upool.tile([P, F], mybir.dt.float32)
                    nc.vector.tensor_add(u[:], v[:], xs)
                    spike_eng.tensor_scalar(
                        ss, u[:], V_TH, None, mybir.AluOpType.is_ge
                    )
                    nc.vector.scalar_tensor_tensor(
                        v[:], u[:], V_TH, u[:],
                        mybir.AluOpType.is_lt, mybir.AluOpType.mult,
                    )
                nc.sync.dma_start(yr[:, blk * tb:(blk + 1) * tb, :], st[:])
    nc.compile()
    return nc


def _get_nc():
    if "nc" not in _NC_CACHE:
        _NC_CACHE["nc"] = build_nc()
    return _NC_CACHE["nc"]


def run_sharded(x_seq, trace=False, nc=None, **kwargs):
    if nc is None:
        nc = _get_nc()
    x2 = np.ascontiguousarray(np.asarray(x_seq, dtype=np.float32)).reshape(T, B * N)
    in_maps = [
        {"x": np.ascontiguousarray(x2[:, c * PERCORE:(c + 1) * PERCORE])}
        for c in range(NCORES)
    ]
    res = run_bass_kernel_spmd(nc, in_maps, list(range(NCORES)), trace=trace, **kwargs)
    out = np.concatenate(
        [np.asarray(res.results[c]["y"]) for c in range(NCORES)], axis=1
    )
    return out.reshape(T, B, N).astype(np.float32, copy=False), res


def kernel(x_seq):
    out, _ = run_sharded(x_seq)
    return out


# revision 11
# speedup vs baseline: 3.4316x; 1.1410x over previous
"""IF spiking-neuron scan (charge / fire / hard-reset) on 8 Trainium2 cores.

Reference recurrence over t (elementwise on every [B, N] element):
    v = v + x_t
    s = (v - 1.0 >= 0)          # spike, 0.0/1.0
    v = (1 - s) * v             # hard reset to 0

Sharding: pure data parallel over the B*N = 262144 element dimension;
each of the 8 cores owns 32768 element chains [T=64, 32768] with zero
communication. Per core the elements live in SBUF as a [128, 256] f32
state tile; the 64-step scan runs locally.

Engine split per timestep (per core):
  DVE   : u = v + x_t            (tensor_tensor add)
          v = (u < 1) * u        (scalar_tensor_tensor fused compare+mult)
  GPSIMD: s = (u >= 1)           (tensor_scalar is_ge -> output tile)
The v-chain (add -> reset) stays on one engine so the serial dependency
never pays a cross-engine semaphore hop; the spike compare branches off.
DMA: timesteps are blocked 8-at-a-time into 1 MiB HWDGE transfers.
"""

import numpy as np

import concourse.bass as bass
import concourse.tile as tile
from concourse import bacc, mybir
from concourse.bass_utils import run_bass_kernel_spmd

T = 64
B = 32
N = 8192
NCORES = 8
PERCORE = (B * N) // NCORES  # 32768 element chains per core
P = 128                      # SBUF partitions
F = PERCORE // P             # 256 elements per partition
TB = 8                       # timesteps per DMA block
NBLK = T // TB

V_TH = 1.0

_NC_CACHE = {}


def build_nc(spike_engine="vector", u_bufs=4, tb=TB):
    nblk = T // tb
    # Bacc (not raw Bass): its compile() splits multi-wait sync conditions
    # into nop/event-semaphore prefixes — walrus accepts at most one sync
    # wait per hardware instruction.
    nc = bacc.Bacc("TRN2", target_bir_lowering=False, debug=False)
    x = nc.dram_tensor("x", [T, PERCORE], mybir.dt.float32, kind="ExternalInput").ap()
    y = nc.dram_tensor("y", [T, PERCORE], mybir.dt.float32, kind="ExternalOutput").ap()

    # [T, P*F] -> [P, T, F]: per partition, each timestep is a contiguous
    # 1 KiB run in DRAM.
    xr = x.rearrange("t (p f) -> p t f", p=P)
    yr = y.rearrange("t (p f) -> p t f", p=P)

    H = F // 2  # two interleaved element streams of 128 columns each
    with tile.TileContext(nc) as tc:
        with (
            tc.tile_pool(name="xin", bufs=3) as xpool,
            tc.tile_pool(name="sout", bufs=nblk) as spool,
            # u values for a whole block accumulate here so one tensor_scalar
            # computes all tb timesteps' spikes at once
            tc.tile_pool(name="ub", bufs=3) as ubpool,
            tc.tile_pool(name="v", bufs=1) as vpool,
        ):
            v = vpool.tile([P, F], mybir.dt.float32)
            nc.vector.memset(v[:], 0.0)
            for blk in range(nblk):
                xt = xpool.tile([P, tb * F], mybir.dt.float32)
                nc.sync.dma_start(xt[:], xr[:, blk * tb:(blk + 1) * tb, :])
                ub = ubpool.tile([P, tb * F], mybir.dt.float32)
                for ti in range(tb):
                    # Interleave two independent half-width chains (A/B):
                    # each op's producer is two instructions back, so the
                    # DVE never stalls on its own write-ack latency.
                    for h in range(2):
                        lo = ti * F + h * H
                        nc.vector.tensor_add(
                            ub[:, lo:lo + H], v[:, h * H:(h + 1) * H],
                            xt[:, lo:lo + H],
                        )
                    for h in range(2):
                        lo = ti * F + h * H
                        nc.vector.scalar_tensor_tensor(
                            v[:, h * H:(h + 1) * H], ub[:, lo:lo + H], V_TH,
                            ub[:, lo:lo + H],
                            mybir.AluOpType.is_lt, mybir.AluOpType.mult,
                        )
                st = spool.tile([P, tb * F], mybir.dt.float32)
                nc.vector.tensor_scalar(
                    st[:], ub[:], V_TH, None, mybir.AluOpType.is_ge
                )
                nc.sync.dma_start(yr[:, blk * tb:(blk + 1) * tb, :], st[:])
    nc.compile()
    return nc


def _get_nc():
    if "nc" not in _NC_CACHE:
        _NC_CACHE["nc"] = build_nc()
    return _NC_CACHE["nc"]


def run_sharded(x_seq, trace=False, nc=None, **kwargs):
    if nc is None:
        nc = _get_nc()
    x2 = np.ascontiguousarray(np.asarray(x_seq, dtype=np.float32)).reshape(T, B * N)
    in_maps = [
        {"x": np.ascontiguousarray(x2[:, c * PERCORE:(c + 1) * PERCORE])}
        for c in range(NCORES)
    ]
    res = run_bass_kernel_spmd(nc, in_maps, list(range(NCORES)), trace=trace, **kwargs)
    out = np.concatenate(
        [np.asarray(res.results[c]["y"]) for c in range(NCORES)], axis=1
    )
    return out.reshape(T, B, N).astype(np.float32, copy=False), res


def kernel(x_seq):
    out, _ = run_sharded(x_seq)
    return out


# revision 12
# speedup vs baseline: 3.6458x; 1.0624x over previous
"""IF spiking-neuron scan (charge / fire / hard-reset) on 8 Trainium2 cores.

Reference recurrence over t (elementwise on every [B, N] element):
    v = v + x_t
    s = (v - 1.0 >= 0)          # spike, 0.0/1.0
    v = (1 - s) * v             # hard reset to 0

Sharding: pure data parallel over the B*N = 262144 element dimension;
each of the 8 cores owns 32768 element chains [T=64, 32768] with zero
communication. Per core the elements live in SBUF as a [128, 256] f32
state tile; the 64-step scan runs locally. All arithmetic is fp32 and
bit-exact vs the reference (adds, compares, and mult-by-0/1 only).

Kernel structure (all compute on the DVE):
  per timestep, two interleaved half-width streams (A/B) so each op's
  producer is two instructions back and the DVE write-ack latency is
  hidden:
      u[A] = v[A] + x_t[A]        (tensor_tensor add)
      u[B] = v[B] + x_t[B]
      v[A] = (u[A] < 1) * u[A]    (scalar_tensor_tensor fused cmp+mult)
      v[B] = (u[B] < 1) * u[B]
  per block of timesteps, ONE tensor_scalar computes every spike at
  once (2x DVE mode) and writes uint8 0/1 directly:
      s[:] = (u_block >= 1)       -> uint8 spike block
  Spikes travel to DRAM as uint8 in [partition, t, f] layout (2 KiB
  contiguous per partition per block -> line-rate DMA, 4x less output
  traffic); the host casts back to float32. Timestep blocks are small
  at the start/end of the scan to shrink the pipeline fill/drain.
"""

import numpy as np

import concourse.bass as bass
import concourse.tile as tile
from concourse import bacc, mybir
from concourse.bass_utils import run_bass_kernel_spmd

T = 64
B = 32
N = 8192
NCORES = 8
PERCORE = (B * N) // NCORES  # 32768 element chains per core
P = 128                      # SBUF partitions
F = PERCORE // P             # 256 elements per partition
H = F // 2                   # half-width for the two interleaved streams

V_TH = 1.0

# timestep block sizes: small at the edges to cut pipeline fill/drain
BLOCKS = [2, 2, 4] + [8] * 6 + [4, 2, 2]
assert sum(BLOCKS) == T

_NC_CACHE = {}


def build_nc(blocks=None):
    blocks = list(BLOCKS if blocks is None else blocks)
    # Bacc (not raw Bass): its compile() splits multi-wait sync conditions
    # into nop/event-semaphore prefixes — walrus accepts at most one sync
    # wait per hardware instruction.
    nc = bacc.Bacc("TRN2", target_bir_lowering=False, debug=False)
    x = nc.dram_tensor("x", [T, PERCORE], mybir.dt.float32, kind="ExternalInput").ap()
    y = nc.dram_tensor("y", [P, T, F], mybir.dt.uint8, kind="ExternalOutput").ap()

    # x: [T, P*F] -> [P, T, F]; per partition each timestep is a contiguous
    # 1 KiB run in DRAM. y is already [P, T, F]: per partition a block of
    # timesteps is one contiguous run.
    xr = x.rearrange("t (p f) -> p t f", p=P)

    with tile.TileContext(nc) as tc:
        with (
            tc.tile_pool(name="xin", bufs=4) as xpool,
            tc.tile_pool(name="sout", bufs=4) as spool,
            tc.tile_pool(name="ub", bufs=3) as ubpool,
            tc.tile_pool(name="v", bufs=1) as vpool,
        ):
            v = vpool.tile([P, F], mybir.dt.float32)
            nc.vector.memset(v[:], 0.0)
            t0 = 0
            for tb in blocks:
                xt = xpool.tile([P, tb * F], mybir.dt.float32, tag="xin")
                nc.sync.dma_start(xt[:], xr[:, t0:t0 + tb, :])
                ub = ubpool.tile([P, tb * F], mybir.dt.float32, tag="ub")
                for ti in range(tb):
                    for h in range(2):
                        lo = ti * F + h * H
                        nc.vector.tensor_add(
                            ub[:, lo:lo + H], v[:, h * H:(h + 1) * H],
                            xt[:, lo:lo + H],
                        )
                    for h in range(2):
                        lo = ti * F + h * H
                        nc.vector.scalar_tensor_tensor(
                            v[:, h * H:(h + 1) * H], ub[:, lo:lo + H], V_TH,
                            ub[:, lo:lo + H],
                            mybir.AluOpType.is_lt, mybir.AluOpType.mult,
                        )
                st = spool.tile([P, tb * F], mybir.dt.uint8, tag="sout")
                nc.vector.tensor_scalar(
                    st[:], ub[:], V_TH, None, mybir.AluOpType.is_ge
                )
                nc.sync.dma_start(y[:, t0:t0 + tb, :], st[:])
                t0 += tb
    nc.compile()
    return nc


def _get_nc():
    if "nc" not in _NC_CACHE:
        _NC_CACHE["nc"] = build_nc()
    return _NC_CACHE["nc"]


def run_sharded(x_seq, trace=False, nc=None, **kwargs):
    if nc is None:
        nc = _get_nc()
    x2 = np.ascontiguousarray(np.asarray(x_seq, dtype=np.float32)).reshape(T, B * N)
    in_maps = [
        {"x": np.ascontiguousarray(x2[:, c * PERCORE:(c + 1) * PERCORE])}
        for c in range(NCORES)
    ]
    res = run_bass_kernel_spmd(nc, in_maps, list(range(NCORES)), trace=trace, **kwargs)
    out = np.empty((T, B * N), dtype=np.float32)
    for c in range(NCORES):
        yc = np.asarray(res.results[c]["y"])          # [P, T, F] uint8
        out[:, c * PERCORE:(c + 1) * PERCORE] = (
            yc.transpose(1, 0, 2).reshape(T, PERCORE)
        )
    return out.reshape(T, B, N), res


def kernel(x_seq):
    out, _ = run_sharded(x_seq)
    return out


# revision 16
# speedup vs baseline: 3.9786x; 1.0913x over previous
"""IF spiking-neuron scan (charge / fire / hard-reset) on 8 Trainium2 cores.

Reference recurrence over t (elementwise on every [B, N] element):
    v = v + x_t
    s = (v - 1.0 >= 0)          # spike, 0.0/1.0
    v = (1 - s) * v             # hard reset to 0

Sharding: pure data parallel over the B*N = 262144 element dimension;
each of the 8 cores owns 32768 element chains [T=64, 32768] with zero
communication. Per core the elements live in SBUF as a [128, 256] f32
state tile; the 64-step scan runs locally. All arithmetic is fp32 and
bit-exact vs the reference (adds, compares, and mult-by-0/1 only).

Kernel structure (all compute on the DVE):
  per timestep, two interleaved half-width streams (A/B) so each op's
  producer is two instructions back and the DVE write-ack latency is
  hidden:
      u[A] = v[A] + x_t[A]        (tensor_tensor add)
      u[B] = v[B] + x_t[B]
      v[A] = (u[A] < 1) * u[A]    (scalar_tensor_tensor fused cmp+mult)
      v[B] = (u[B] < 1) * u[B]
  per block of timesteps, ONE tensor_scalar computes every spike at
  once (2x DVE mode) and writes uint8 0/1 directly:
      s[:] = (u_block >= 1)       -> uint8 spike block
  Spikes travel to DRAM as uint8 in [partition, t, f] layout (2 KiB
  contiguous per partition per block -> line-rate DMA, 4x less output
  traffic); the host casts back to float32. Timestep blocks are small
  at the start/end of the scan to shrink the pipeline fill/drain.
"""

import numpy as np

import concourse.bass as bass
import concourse.tile as tile
from concourse import bacc, mybir
from concourse.bass_utils import run_bass_kernel_spmd

T = 64
B = 32
N = 8192
NCORES = 8
PERCORE = (B * N) // NCORES  # 32768 element chains per core
P = 128                      # SBUF partitions
F = PERCORE // P             # 256 elements per partition
H = F // 2                   # half-width for the two interleaved streams

V_TH = 1.0

# timestep block sizes: small at the edges to cut pipeline fill/drain
BLOCKS = [2, 2, 4] + [8] * 6 + [4, 2, 2]
assert sum(BLOCKS) == T

_NC_CACHE = {}


def build_nc(blocks=None, spike_on_act=True):
    blocks = list(BLOCKS if blocks is None else blocks)
    # Bacc (not raw Bass): its compile() splits multi-wait sync conditions
    # into nop/event-semaphore prefixes — walrus accepts at most one sync
    # wait per hardware instruction.
    nc = bacc.Bacc("TRN2", target_bir_lowering=False, debug=False)
    x = nc.dram_tensor("x", [T, PERCORE], mybir.dt.float32, kind="ExternalInput").ap()
    y = nc.dram_tensor("y", [P, T, F], mybir.dt.uint8, kind="ExternalOutput").ap()

    # x: [T, P*F] -> [P, T, F]; per partition each timestep is a contiguous
    # 1 KiB run in DRAM. y is already [P, T, F]: per partition a block of
    # timesteps is one contiguous run.
    xr = x.rearrange("t (p f) -> p t f", p=P)

    with tile.TileContext(nc) as tc:
        with (
            tc.tile_pool(name="xin", bufs=4) as xpool,
            tc.tile_pool(name="sout", bufs=4) as spool,
            tc.tile_pool(name="ub", bufs=3) as ubpool,
            tc.tile_pool(name="zb", bufs=2) as zpool,
            tc.tile_pool(name="v", bufs=1) as vpool,
        ):
            v = vpool.tile([P, F], mybir.dt.float32)
            nc.vector.memset(v[:], 0.0)
            t0 = 0
            for tb in blocks:
                xt = xpool.tile([P, tb * F], mybir.dt.float32, tag="xin")
                nc.sync.dma_start(xt[:], xr[:, t0:t0 + tb, :])
                ub = ubpool.tile([P, tb * F], mybir.dt.float32, tag="ub")
                for ti in range(tb):
                    for h in range(2):
                        lo = ti * F + h * H
                        nc.vector.tensor_add(
                            ub[:, lo:lo + H], v[:, h * H:(h + 1) * H],
                            xt[:, lo:lo + H],
                        )
                    for h in range(2):
                        lo = ti * F + h * H
                        nc.vector.scalar_tensor_tensor(
                            v[:, h * H:(h + 1) * H], ub[:, lo:lo + H], V_TH,
                            ub[:, lo:lo + H],
                            mybir.AluOpType.is_lt, mybir.AluOpType.mult,
                        )
                st = spool.tile([P, tb * F], mybir.dt.uint8, tag="sout")
                if spike_on_act:
                    # Spike path on the otherwise-idle scalar engine, exact
                    # even when u == V_TH:  z = sign(V_TH - u) in {-1,0,1},
                    # r = relu(z) in {0,1}; r == 1 - s, host flips it back.
                    zt = zpool.tile([P, tb * F], mybir.dt.float32, tag="zb")
                    nc.scalar.activation(
                        zt[:], ub[:], mybir.ActivationFunctionType.Sign,
                        bias=V_TH, scale=-1.0,
                    )
                    nc.scalar.activation(
                        st[:], zt[:], mybir.ActivationFunctionType.Relu,
                    )
                else:
                    nc.vector.tensor_scalar(
                        st[:], ub[:], V_TH, None, mybir.AluOpType.is_ge
                    )
                nc.sync.dma_start(y[:, t0:t0 + tb, :], st[:])
                t0 += tb
    nc.compile()
    return nc


def _get_nc():
    if "nc" not in _NC_CACHE:
        _NC_CACHE["nc"] = build_nc()
    return _NC_CACHE["nc"]


def run_sharded(x_seq, trace=False, nc=None, spike_on_act=True, **kwargs):
    if nc is None:
        nc = _get_nc()
    x2 = np.ascontiguousarray(np.asarray(x_seq, dtype=np.float32)).reshape(T, B * N)
    in_maps = [
        {"x": np.ascontiguousarray(x2[:, c * PERCORE:(c + 1) * PERCORE])}
        for c in range(NCORES)
    ]
    res = run_bass_kernel_spmd(nc, in_maps, list(range(NCORES)), trace=trace, **kwargs)
    out = np.empty((T, B * N), dtype=np.float32)
    for c in range(NCORES):
        yc = np.asarray(res.results[c]["y"])          # [P, T, F] uint8
        r = yc.transpose(1, 0, 2).reshape(T, PERCORE)
        # device stores r = 1 - s on the spike_on_act path
        out[:, c * PERCORE:(c + 1) * PERCORE] = (1 - r) if spike_on_act else r
    return out.reshape(T, B, N), res


def kernel(x_seq):
    out, _ = run_sharded(x_seq)
    return out
